# revision 50
# baseline (speedup 1.0000x reference)
"""Soft decision-tree forward kernel for Trainium2 (8 NeuronCores, SPMD).

Per core (16384 rows, 128 row-tiles of 128), fp16 data, f32 accumulation.
Pipeline over 8 slabs of 16 tiles (2048 rows), 2 groups of 8 tiles each:

  1. z06 = xt-tile^T @ G06            (PE, 8 mm/group, K=33, N=128)
  2. act06 = sigmoid(z06)             (ACT, 1 op/group over [128, 1024])
  3. z7T = G7^T @ xt-cols             (PE, node-major, 1 mm/group N=1024)
  4. sig7 = sigmoid(z7T)              (ACT, 1 op/group; threshold baked
                                       into G7's ones-row)
  5. tree DP levels 1..6, batch-major (Pool levels 1-4, DVE levels 5-6)
  6. P7 tile -> P7T via SBUF->SBUF xbar transpose DMA (SP queue, per tile)
  7. r7t = P7T * sig7                 (DVE, node-major)
  8. pT = A^T@P7T + B^T@r7t          (PE, 512-col chunks, 4 quadrant-packed
                                       per [128,512] PSUM tile)
  9. PSUM -> out_sb (Pool), one final DMA [128, 4096] -> DRAM

Node layout: level d's column k <-> heap node bitrev_d(k); baked into
G06 / G7 / A / B host-side.  All batch ordering is PLAIN b-order:
transposed P7T columns, sig7 columns and final pT columns all use
b = tile*128 + row, so no input pre-permute (xQ) and no host
un-permute are needed.  Output pT_sb[32q+c, s*512+j] = p(class c) for
row b = (4s+q)*512 + j.
"""

import sys

sys.path.insert(0, "/opt/trn_rl_repo")

import numpy as np

import concourse.bacc as bacc
import concourse.bass as bass
import concourse.mybir as mybir
import concourse.tile as tile
from concourse import bass_utils

# ---- problem constants (hardcoded per contract) ----
BATCH = 131072
N_FEAT = 32
N_CLASSES = 10
N_CORES = 8
R = BATCH // N_CORES          # 16384 rows per core
TILE = 128
N_TILES = R // TILE           # 128
SLAB = 16                     # tiles per slab (DP granularity)
N_SLAB = N_TILES // SLAB      # 8
GRP = 8                       # tiles per group (PSUM/ACT granularity)
KDIM = N_FEAT + 1             # 33
FIN_SUB = 512                 # final matmul chunk (one PSUM bank, 4/slab)

F32 = mybir.dt.float32
F16 = mybir.dt.float16
SIGMOID = mybir.ActivationFunctionType.Sigmoid

_COMPILED = None


def _bitrev(k, bits):
    r = 0
    for _ in range(bits):
        r = (r << 1) | (k & 1)
        k >>= 1
    return r


def _host_prep(thresholds, feats, leaf_class):
    """G06 [33,128], G7 [33,128], A/B [128,10] in device layout."""
    G06 = np.zeros((KDIM, 128), dtype=np.float32)
    f0, t0 = int(feats[0]), float(thresholds[0])
    G06[f0, 0] = -1.0
    G06[N_FEAT, 0] = +t0
    G06[f0, 1] = +1.0
    G06[N_FEAT, 1] = -t0
    for d in range(1, 7):
        n = 1 << d
        start = n - 1
        for k in range(n):
            j = _bitrev(k, d)
            G06[int(feats[start + j]), n + k] = 1.0
            G06[N_FEAT, n + k] = -float(thresholds[start + j])
    G7 = np.zeros((KDIM, 128), dtype=np.float32)
    start7 = 127
    for k in range(128):
        j = _bitrev(k, 7)
        G7[int(feats[start7 + j]), k] = 1.0
        G7[N_FEAT, k] = -float(thresholds[start7 + j])
    Lc = np.empty(128, dtype=np.int64)
    Rc = np.empty(128, dtype=np.int64)
    for k in range(128):
        j7 = _bitrev(k, 7)
        Lc[k] = leaf_class[2 * j7]
        Rc[k] = leaf_class[2 * j7 + 1]
    A = np.zeros((128, N_CLASSES), dtype=np.float32)
    Bm = np.zeros((128, N_CLASSES), dtype=np.float32)
    A[np.arange(128), Lc] = 1.0
    Bm[np.arange(128), Rc] += 1.0
    Bm[np.arange(128), Lc] -= 1.0
    return G06, G7, A, Bm


def _build_program():
    nc = bacc.Bacc("TRN2", target_bir_lowering=False, debug=False,
                   num_devices=N_CORES)

    xt_d = nc.dram_tensor("xT", [KDIM, R], F16, kind="ExternalInput")
    # packed consts: cols 0:128 G06, 128:256 G7 (rows 0:33), 256:266 A,
    # 266:276 B (rows 0:128)
    c_d = nc.dram_tensor("C", [128, 276], F16, kind="ExternalInput")
    pt_d = nc.dram_tensor("pT", [128, N_TILES * N_CLASSES], F16,
                          kind="ExternalOutput")

    with tile.TileContext(nc) as tc:
        with (
            tc.tile_pool(name="const", bufs=1) as cpool,
            tc.tile_pool(name="act", bufs=4) as act_pool,
            tc.tile_pool(name="tree", bufs=2) as tree_pool,
            tc.tile_pool(name="p7", bufs=2) as p7_pool,
            tc.tile_pool(name="p7t", bufs=5) as p7t_pool,
            tc.tile_pool(name="sig", bufs=4) as sig_pool,
            tc.tile_pool(name="r7t", bufs=10) as r7t_pool,
            tc.tile_pool(name="out", bufs=1) as out_pool,
            tc.tile_pool(name="zpsum", bufs=3, space="PSUM") as zpsum,
            tc.tile_pool(name="fpsum", bufs=2, space="PSUM") as fpsum,
        ):
            consts = cpool.tile([128, 276], F16, tag="C")
            nc.scalar.dma_start(consts[:], c_d.ap()[:, :])
            g06 = consts[0:KDIM, 0:128]
            g7 = consts[0:KDIM, 128:256]
            a_s = consts[:, 256:266]
            b_s = consts[:, 266:276]
            # x loads: small leading chunks so compute starts early
            SLABC = SLAB * TILE  # 2048
            chunk_cols = [SLABC, SLABC, 2 * SLABC, 2 * SLABC, 2 * SLABC]
            slab_part = [0, 1, 2, 2, 3, 3, 4, 4]
            slab_base = [0, 0, 0, SLABC, 0, SLABC, 0, SLABC]
            xt_parts = []
            off = 0
            for c, cols in enumerate(chunk_cols):
                xt_p = cpool.tile([KDIM, cols], F16, tag=f"xTp{c}",
                                  name=f"xTp{c}")
                nc.sync.dma_start(xt_p[:], xt_d.ap()[:, off:off + cols])
                xt_parts.append(xt_p)
                off += cols

            out_sb = out_pool.tile([128, N_TILES * N_CLASSES], F16,
                                   tag="out")

            p7t_slabs = {}
            r7ts = {}

            def emit_finals(g):
                """out[b,c] = P7T-tile^T @ A + R7T-tile^T @ B for group g.

                P7T/R7T tiles are the (LDW-pipe) stationary; the tiny A/B
                are moving, so each matmul streams only 10 columns."""
                s, h = divmod(g, 2)
                fin = fpsum.tile([128, GRP, N_CLASSES], F32, tag="fin",
                                 name="fin")
                p7t = p7t_slabs[s]
                r7t = r7ts.pop(g)
                for t in range(GRP):
                    out_sl = fin[:, t, :]
                    nc.tensor.matmul(out_sl, p7t[:, h * GRP + t, :], a_s,
                                     start=True, stop=False)
                    nc.tensor.matmul(out_sl, r7t[:, t, :], b_s,
                                     start=False, stop=True)
                if h == 1:
                    del p7t_slabs[s]
                dst = out_sb[:, g * GRP * N_CLASSES:(g + 1) * GRP * N_CLASSES]
                if g % 2 == 0:
                    nc.vector.tensor_copy(dst, fin[:, :, :])
                else:
                    nc.scalar.copy(dst, fin[:, :, :])

            for s in range(N_SLAB):
                xt_p = xt_parts[slab_part[s]]
                act_sl = act_pool.tile([TILE, SLAB, 128], F16, tag="act06",
                                       name="act06")
                sig_tiles = []
                for h in range(2):
                    base = slab_base[s] + h * GRP * TILE
                    z06 = zpsum.tile([TILE, GRP * 128], F32, tag="z",
                                     name="z06")
                    for t in range(GRP):
                        nc.tensor.matmul(
                            z06[:, bass.ts(t, 128)],
                            xt_p[:, base + t * TILE:base + (t + 1) * TILE],
                            g06, start=True, stop=True)
                    nc.scalar.activation(
                        act_sl[:, bass.ts(h, GRP), :], z06[:], SIGMOID)
                    z7 = zpsum.tile([TILE, GRP * 128], F32, tag="z",
                                    name="z7")
                    for i in range(2):
                        nc.tensor.matmul(
                            z7[:, bass.ts(i, 512)], g7,
                            xt_p[:, base + i * 512:base + (i + 1) * 512],
                            start=True, stop=True)
                    sig7 = sig_pool.tile([TILE, GRP, TILE], F16, tag="sig7",
                                         name="sig7")
                    nc.scalar.activation(sig7[:], z7[:], SIGMOID)
                    sig_tiles.append(sig7)

                # --- tree DP levels 1..6 (Pool: 1-4, DVE: 5-6) ---
                prev = act_sl[:, :, 0:2]
                for d in range(1, 7):
                    n = 1 << d
                    if d < 6:
                        cur = tree_pool.tile([TILE, SLAB, 2 * n], F16,
                                             tag=f"P{d + 1}", name=f"P{d + 1}")
                    else:
                        cur = p7_pool.tile([TILE, SLAB, 128], F16,
                                           tag="P7", name="P7")
                    eng = nc.gpsimd if d <= 4 else nc.vector
                    eng.tensor_mul(
                        cur[:, :, n:2 * n], prev[:], act_sl[:, :, n:2 * n])
                    eng.tensor_sub(
                        cur[:, :, 0:n], prev[:], cur[:, :, n:2 * n])
                    prev = cur[:, :, :]

                # --- direct SBUF->SBUF xbar transpose ---
                # in P7 [p, (t,n)] flat cols i=t*128+n; out[q,j,r]=in[r,j*128+q]
                # => p7t[n, t, p], flat columns are plain b = t*128 + p
                p7t = p7t_pool.tile([TILE, SLAB, TILE], F16, tag="p7t",
                                    name="p7t")
                nc.sync.dma_start_transpose(p7t[:, :, :], prev[:])
                p7t_slabs[s] = p7t

                # --- r7t = p7t * sig7 (node-major) ---
                for h in range(2):
                    r7t = r7t_pool.tile([TILE, GRP, TILE], F16, tag="r7t",
                                        name="r7t")
                    nc.vector.tensor_mul(
                        r7t[:], p7t[:, bass.ts(h, GRP), :],
                        sig_tiles[h][:])
                    r7ts[2 * s + h] = r7t

                if s >= 1:
                    emit_finals(2 * (s - 1))
                    emit_finals(2 * (s - 1) + 1)
                if s == N_SLAB - 1:
                    # first output half: drains for slabs 0..3 are done
                    half = N_TILES * N_CLASSES // 2
                    nc.sync.dma_start(pt_d.ap()[:, :half], out_sb[:, :half])

            for g in range(2 * (N_SLAB - 1), 2 * N_SLAB):
                emit_finals(g)
            half = N_TILES * N_CLASSES // 2
            nc.sync.dma_start(pt_d.ap()[:, half:], out_sb[:, half:])

    nc.compile()
    return nc


def _get_compiled():
    global _COMPILED
    if _COMPILED is None:
        _COMPILED = _build_program()
    return _COMPILED


def kernel(x, thresholds, feats, leaf_class, _trace=False):
    x = np.asarray(x, dtype=np.float32)
    thresholds = np.asarray(thresholds, dtype=np.float32)
    feats = np.asarray(feats, dtype=np.int32)
    leaf_class = np.asarray(leaf_class, dtype=np.int32)
    assert x.shape == (BATCH, N_FEAT)

    G06, G7, A, Bm = _host_prep(thresholds, feats, leaf_class)
    f16 = np.float16

    x_ext_T = np.empty((KDIM, BATCH), dtype=f16)
    x_ext_T[:N_FEAT, :] = x.T.astype(f16)
    x_ext_T[N_FEAT, :] = 1.0

    consts = np.zeros((128, 276), dtype=f16)
    consts[:KDIM, 0:128] = G06
    consts[:KDIM, 128:256] = G7
    consts[:, 256:266] = A
    consts[:, 266:276] = Bm

    in_maps = []
    for c in range(N_CORES):
        sl = slice(c * R, (c + 1) * R)
        in_maps.append({
            "xT": np.ascontiguousarray(x_ext_T[:, sl]),
            "C": consts,
        })

    nc = _get_compiled()
    res = bass_utils.run_bass_kernel_spmd(
        nc, in_maps, core_ids=list(range(N_CORES)),
        trace=_trace, trace_cores=[0] if _trace else None,
    )

    out = np.empty((BATCH, N_CLASSES), dtype=np.float32)
    for c in range(N_CORES):
        sl = slice(c * R, (c + 1) * R)
        pt = res.results[c]["pT"].astype(np.float32)  # [128, 128*10]
        # pt[p, tile*10 + cc] = p(class cc) of row b = tile*128 + p
        v = pt.reshape(128, N_TILES, N_CLASSES)       # [p, tile, cc]
        out[sl] = v.transpose(1, 0, 2).reshape(R, N_CLASSES)
    if _trace:
        kernel._last_results = res
    return out
